# revision 9
# baseline (speedup 1.0000x reference)
"""ChebNet (K=3, 2 layers) message-passing kernel for 8 Trainium2 NeuronCores.

Strategy (dest-sharded; batched dma_gather + CCE dma_scatter_add):
  - Nodes are sharded across 8 cores by contiguous id range (12500 each).
  - Within a core, destination nodes are sorted by in-degree (descending):
    rank r -> grid slot r (column b = r//128, partition p = r%128).
  - The propagation  L_hat @ t = -D^-1/2 A D^-1/2 t  is factored as
    -d * (A @ (d * t)): only an unweighted gather+sum over edges remains;
    scalings/signs fold into per-node d multiplies and host-prepared weights.
  - A @ ts per core: edge source rows are fetched with `dma_gather` (one
    descriptor per edge, int16 indices into one of 4 windows of the
    replicated fp32 table) into SBUF chunk buffers, then `dma_scatter_add`
    CCE-accumulates each edge row into the destination accumulator in SBUF
    (parity-split layout: even columns in half A, odd in half B).  Scatter
    calls are split by edge occurrence within (dest, window) so no
    destination repeats inside a call (CCE read-modify-write stays
    race-free).  ~25 gather + ~60 scatter calls per propagation replace the
    ~1200 per-column indirect-DMA chains of the old scheme (whose per-call
    SWDGE overhead dominated at ~7 us/call).
  - Full scaled node features are replicated on every core via AllGather
    between propagations (fp32 tables; dma_gather requires 256 B rows).
  - ChebConv matmuls run in bf16 with K packed to 128 ([T0;T1] against
    [W0-W2;-W1]) plus a K=64 term (2*W2), fp32 PSUM accumulation.
  - Output is written feature-major [COUT, PADLOC]; the host transposes.

Self-contained: hardcodes the problem shapes; builds indices from the given
edge_index on the host (numpy), compiles one SPMD Bass program, runs it on
cores 0..7, and reassembles the full [100000, 32] output.
"""

import numpy as np

N_NODES = 100_000
N_EDGES = 1_200_000
CIN, CHID, COUT, KCH = 64, 64, 32, 3
NCORES = 8
NLOC = N_NODES // NCORES            # 12500
NCOLS = (NLOC + 127) // 128         # 98
PADLOC = NCOLS * 128                # 12544
TSTRIDE = PADLOC + 16               # per-core block in gather tables
NTAB = TSTRIDE * NCORES             # 100480
WIN = 2 * TSTRIDE                   # 25120 rows: int16-addressable window
NWIN = 4
ZLOC = PADLOC                       # zero row inside each window (block pad)
DUMP = PADLOC                       # dest slots >= DUMP land in dump column
CH = 4096                           # max slots per gather chunk / scatter run
GH = NCOLS // 2 + 1                 # acc half-columns incl dump (50)

_CACHE = {}


def _bf16(a):
    import ml_dtypes
    return np.asarray(a, dtype=ml_dtypes.bfloat16)


def _wrap16(vals):
    """int16 index list (len%16==0) -> [128, n/16] wrapped+replicated."""
    n = len(vals)
    assert n % 16 == 0
    return np.tile(vals.reshape(n // 16, 16).T, (8, 1))


# --------------------------------------------------------------------------
# Host-side index construction
# --------------------------------------------------------------------------

def _prep_indices(edge_index):
    row = np.asarray(edge_index[0], dtype=np.int64)
    col = np.asarray(edge_index[1], dtype=np.int64)
    deg = np.bincount(row, minlength=N_NODES)
    d = np.where(deg > 0, 1.0 / np.sqrt(np.maximum(deg, 1)), 0.0).astype(np.float32)

    # per-core degree-descending permutation
    rank = np.empty(N_NODES, dtype=np.int64)
    lnode = np.empty((NCORES, NLOC), dtype=np.int64)
    for c in range(NCORES):
        ids = np.arange(c * NLOC, (c + 1) * NLOC)
        order = np.argsort(-deg[ids], kind="stable")
        lnode[c] = ids[order]
        rank[ids[order]] = np.arange(NLOC)
    pos = (np.arange(N_NODES) // NLOC) * TSTRIDE + rank   # table position

    # per-edge: dest core/rank, source window/local row
    dpos = pos[row]
    core_e = dpos // TSTRIDE
    r_e = dpos % TSTRIDE                   # dest rank (< NLOC)
    spos = pos[col]
    w_e = spos // WIN
    s_e = (spos % WIN).astype(np.int64)    # source row within window

    # window-degree profile per (core, window, rank); schedule = max over cores
    # n_cnt[c][w][k] = #dests on core c with window-degree > k
    wdeg = np.zeros((NCORES, NWIN, NLOC), dtype=np.int64)
    for c in range(NCORES):
        for w in range(NWIN):
            m = (core_e == c) & (w_e == w)
            wdeg[c, w] = np.bincount(r_e[m], minlength=NLOC)
    maxk = int(wdeg.max())
    # counts[c, w, k]
    counts = np.zeros((NCORES, NWIN, maxk), dtype=np.int64)
    for c in range(NCORES):
        for w in range(NWIN):
            for k in range(maxk):
                counts[c, w, k] = int((wdeg[c, w] > k).sum())
    n_sched = counts.max(axis=0)           # [NWIN, maxk]

    # run pieces per (w, k): lengths (%128, split at CH)
    # piece list per window, in k order; then chunks greedy-pack pieces <= CH
    def r128(x):
        return (x + 127) // 128 * 128

    pieces = {w: [] for w in range(NWIN)}  # (k, piece_off_in_run, L)
    for w in range(NWIN):
        for k in range(maxk):
            L = r128(int(n_sched[w, k]))
            if L == 0:
                continue
            off = 0
            while off < L:
                pieces[w].append((k, off, min(CH, L - off)))
                off += CH

    # chunks: (w, n, [piece...]), slot ranges; gather/scatter col offsets
    sched = []          # list of dicts
    gcols = scols = 0
    slot_base = {}      # (w, k) -> global slot base of the run
    slot_cursor = 0
    run_counter = 0
    for w in range(NWIN):
        cur = None
        for (k, off, L) in pieces[w]:
            if cur is None or cur["n"] + L > CH:
                if cur is not None:
                    sched.append(cur)
                cur = {"w": w, "n": 0, "gc0": gcols, "runs": [],
                       "slot0": slot_cursor}
                gcols_start = gcols
            if off == 0:
                slot_base[(w, k)] = slot_cursor
            cur["runs"].append({"a": cur["n"], "n": L, "sc0": scols,
                                "set": run_counter % 2})
            run_counter += 1
            cur["n"] += L
            gcols += L // 16
            scols += L // 16
            slot_cursor += L
        if cur is not None:
            sched.append(cur)
    nslot = slot_cursor

    # per-core index streams
    gixs = np.full((NCORES, nslot), ZLOC, dtype=np.int16)
    sixs = np.empty((NCORES, nslot), dtype=np.int16)
    sixs[:] = (DUMP + np.arange(nslot) % 128).astype(np.int16)
    for c in range(NCORES):
        for w in range(NWIN):
            m = (core_e == c) & (w_e == w)
            r = r_e[m]
            s = s_e[m]
            o = np.argsort(r, kind="stable")
            r, s = r[o], s[o]
            first = np.ones(len(r), dtype=bool)
            first[1:] = r[1:] != r[:-1]
            gs = np.maximum.accumulate(
                np.where(first, np.arange(len(r)), 0))
            occ = np.arange(len(r)) - gs
            # position within run k = index among qualifying ranks, which is
            # the sort-order of r among edges with the same occ
            o2 = np.lexsort((r, occ))
            r2, s2, occ2 = r[o2], s[o2], occ[o2]
            firstk = np.ones(len(r2), dtype=bool)
            firstk[1:] = occ2[1:] != occ2[:-1]
            ks = np.maximum.accumulate(
                np.where(firstk, np.arange(len(r2)), 0))
            posk = np.arange(len(r2)) - ks
            base = np.array([slot_base[(w, int(kk))] for kk in occ2])
            slots = base + posk
            gixs[c, slots] = s2.astype(np.int16)
            sixs[c, slots] = r2.astype(np.int16)

    # wrap the streams per call
    gw = np.empty((NCORES, 128, nslot // 16), dtype=np.int16)
    sw = np.empty((NCORES, 128, nslot // 16), dtype=np.int16)
    for c in range(NCORES):
        for ck in sched:
            s0, n, gc0 = ck["slot0"], ck["n"], ck["gc0"]
            gw[c][:, gc0:gc0 + n // 16] = _wrap16(gixs[c, s0:s0 + n])
            for rn in ck["runs"]:
                a, nr, sc0 = rn["a"], rn["n"], rn["sc0"]
                sw[c][:, sc0:sc0 + nr // 16] = _wrap16(
                    sixs[c, s0 + a:s0 + a + nr])

    dloc = np.zeros((NCORES, PADLOC), dtype=np.float32)
    for c in range(NCORES):
        dloc[c, :NLOC] = d[lnode[c]]

    return {
        "lnode": lnode, "gw": gw, "sw": sw, "sched": sched, "nslot": nslot,
        "dloc": dloc, "d": d,
    }


# --------------------------------------------------------------------------
# Device program
# --------------------------------------------------------------------------

def _cm(b):
    """column -> acc/tsn split-layout index (evens | odds, dump at 49/99)."""
    return b // 2 if b % 2 == 0 else GH + b // 2


def _build_program(sched, nslot):
    from collections import deque
    from concourse import bass, bacc, tile, mybir

    f32 = mybir.dt.float32
    bf16 = mybir.dt.bfloat16
    i16 = mybir.dt.int16
    nc = bacc.Bacc("TRN2", target_bir_lowering=False, debug=False,
                   num_devices=NCORES)

    ts0 = nc.dram_tensor("ts0", [NTAB, CIN], f32, kind="ExternalInput")
    xfm = nc.dram_tensor("xfm", [CIN, PADLOC], bf16, kind="ExternalInput")
    dnm_in = nc.dram_tensor("dnm2", [128, 2 * GH], f32, kind="ExternalInput")
    gix_in = nc.dram_tensor("gixs", [128, nslot // 16], i16,
                            kind="ExternalInput")
    six_in = nc.dram_tensor("sixs", [128, nslot // 16], i16,
                            kind="ExternalInput")
    w1a_in = nc.dram_tensor("w1a", [128, CHID], bf16, kind="ExternalInput")
    w1b_in = nc.dram_tensor("w1b", [64, CHID], bf16, kind="ExternalInput")
    w2a_in = nc.dram_tensor("w2a", [128, COUT], bf16, kind="ExternalInput")
    w2b_in = nc.dram_tensor("w2b", [64, COUT], bf16, kind="ExternalInput")
    b1_in = nc.dram_tensor("b1v", [CHID, 1], f32, kind="ExternalInput")
    b2_in = nc.dram_tensor("b2v", [COUT, 1], f32, kind="ExternalInput")
    idf_in = nc.dram_tensor("identf", [128, 128], f32, kind="ExternalInput")
    idb_in = nc.dram_tensor("identb", [128, 128], bf16, kind="ExternalInput")
    out_d = nc.dram_tensor("out", [COUT, PADLOC], f32, kind="ExternalOutput")

    Relu = mybir.ActivationFunctionType.Relu
    Ident = mybir.ActivationFunctionType.Identity
    MUL = mybir.AluOpType.mult
    ADD = mybir.AluOpType.add
    BYP = mybir.AluOpType.bypass

    ntile = (PADLOC + 511) // 512    # 25 matmul tiles of 512 nodes

    with tile.TileContext(nc) as tc:
        with (
            tc.tile_pool(name="consts", bufs=1) as consts,
            tc.tile_pool(name="accp", bufs=1) as accp,
            tc.tile_pool(name="fmp", bufs=1) as fmp,
            tc.tile_pool(name="gp", bufs=3) as gp,
            tc.tile_pool(name="fm2p", bufs=3) as fm2p,
            tc.tile_pool(name="ofmp", bufs=3) as ofmp,
            tc.tile_pool(name="psT", bufs=3, space="PSUM") as psT,
            tc.tile_pool(name="psM", bufs=2, space="PSUM") as psM,
            tc.tile_pool(name="psH", bufs=3, space="PSUM") as psH,
            tc.tile_pool(name="dram", bufs=1, space="DRAM") as dram,
        ):
            # ---- constants into SBUF
            gixs_sb = consts.tile([128, nslot // 16], i16)
            nc.sync.dma_start(out=gixs_sb[:], in_=gix_in[:])
            sixs_sb = consts.tile([128, nslot // 16], i16)
            nc.sync.dma_start(out=sixs_sb[:], in_=six_in[:])
            dnm = consts.tile([128, 2 * GH, 1], f32)
            nc.sync.dma_start(out=dnm[:],
                              in_=dnm_in[:].rearrange("p (b o) -> p b o", o=1))
            dnm_bc = dnm[:].to_broadcast([128, 2 * GH, CIN])
            w1a = consts.tile([128, CHID], bf16)
            nc.sync.dma_start(out=w1a[:], in_=w1a_in[:])
            w1b = consts.tile([64, CHID], bf16)
            nc.sync.dma_start(out=w1b[:], in_=w1b_in[:])
            w2a = consts.tile([128, COUT], bf16)
            nc.sync.dma_start(out=w2a[:], in_=w2a_in[:])
            w2b = consts.tile([64, COUT], bf16)
            nc.sync.dma_start(out=w2b[:], in_=w2b_in[:])
            b1v = consts.tile([CHID, 1], f32)
            nc.sync.dma_start(out=b1v[:], in_=b1_in[:])
            b2v = consts.tile([COUT, 1], f32)
            nc.sync.dma_start(out=b2v[:], in_=b2_in[:])
            identf = consts.tile([128, 128], f32)
            nc.sync.dma_start(out=identf[:], in_=idf_in[:])
            identb = consts.tile([128, 128], bf16)
            nc.sync.dma_start(out=identb[:], in_=idb_in[:])
            zrow = consts.tile([16, CIN], f32)
            nc.gpsimd.memset(zrow[:], 0.0)

            # fmA: partitions 0:64 = Tx0 features (x / h), 64:128 = Tx1_s
            fmA = fmp.tile([128, PADLOC], bf16)
            nc.sync.dma_start(out=fmA[0:64, :], in_=xfm[:])
            # split-ordered (evens|odds) staging for AllGather inputs
            tsn = fmp.tile([128, 2 * GH, CIN], f32)
            # accumulator sets, split-ordered; two sets for scatter overlap
            acc0 = accp.tile([128, 2 * GH, CIN], f32)
            acc1 = accp.tile([128, 2 * GH, CIN], f32)

            # DRAM: AllGather bounce + tables
            ag_in = dram.tile([TSTRIDE, CIN], f32, name="ag_in")
            tabs = [dram.tile([NTAB, CIN], f32, name=f"tab{i}",
                              addr_space="Shared") for i in range(3)]
            nc.sync.dma_start(out=ag_in[PADLOC:TSTRIDE, :], in_=zrow[:])

            def prop(table):
                nc.vector.memset(acc0[:], 0.0)
                nc.vector.memset(acc1[:], 0.0)

                def scatters(g, ck):
                    for rn in ck["runs"]:
                        acc = acc0 if rn["set"] == 0 else acc1
                        a, nr, sc0 = rn["a"], rn["n"], rn["sc0"]
                        nc.gpsimd.dma_scatter_add(
                            out_ap=acc[:, 0:GH, :],
                            in_ap=g[:, a // 128:(a + nr) // 128, :],
                            idxs_ap=sixs_sb[:, sc0:sc0 + nr // 16],
                            num_idxs=nr, num_idxs_reg=nr, elem_size=CIN,
                            sbuf_tokens_per_rank=128, parity_reg=0,
                            out_ap_other=acc[:, GH:2 * GH, :],
                            single_packet=False)

                pend = deque()
                for j, ck in enumerate(sched):
                    g = gp.tile([128, CH // 128, CIN], f32, tag="gg",
                                name=f"g_{j}")
                    w, n, gc0 = ck["w"], ck["n"], ck["gc0"]
                    nc.gpsimd.dma_gather(
                        out_ap=g[:, :n // 128, :],
                        in_ap=table[w * WIN:(w + 1) * WIN, :],
                        idxs_ap=gixs_sb[:, gc0:gc0 + n // 16],
                        num_idxs=n, num_idxs_reg=n, elem_size=CIN,
                        single_packet=False)
                    pend.append((g, ck))
                    if len(pend) == 3:
                        scatters(*pend.popleft())
                while pend:
                    scatters(*pend.popleft())
                # combine sets
                nc.vector.tensor_tensor(out=acc0[:], in0=acc0[:],
                                        in1=acc1[:], op=ADD)

            def scale_acc():
                nc.vector.tensor_tensor(out=acc0[:], in0=acc0[:], in1=dnm_bc,
                                        op=MUL)

            def make_tsn():
                nc.vector.tensor_tensor(out=tsn[:], in0=acc0[:], in1=dnm_bc,
                                        op=MUL)

            def fm_fill(dst, dst_part0, tag):
                """acc0 (=t1) columns -> dst[dst_part0:+64] feature-major."""
                for q in range(0, NCOLS, 4):
                    nb = min(4, NCOLS - q)
                    pt = psT.tile([64, 512], f32, tag="pt",
                                  name=f"pt{tag}_{q}")
                    for bi in range(nb):
                        nc.tensor.transpose(out=pt[:, bi * 128:(bi + 1) * 128],
                                            in_=acc0[:, _cm(q + bi), :],
                                            identity=identf[:])
                    nc.vector.tensor_copy(
                        out=dst[dst_part0:dst_part0 + 64,
                                q * 128:(q + nb) * 128],
                        in_=pt[:, :nb * 128])

            def allgather(tab):
                # evens: table rows 2g*128+p ; odds: rows (2g+1)*128+p
                av = ag_in[0:PADLOC, :].rearrange("(g t p) c -> t p g c",
                                                  t=2, p=128)
                nc.sync.dma_start(out=av[0], in_=tsn[:, 0:GH - 1, :])
                nc.sync.dma_start(out=av[1], in_=tsn[:, GH:2 * GH - 1, :])
                nc.gpsimd.collective_compute(
                    "AllGather", BYP,
                    replica_groups=[list(range(NCORES))],
                    ins=[ag_in[:].opt()], outs=[tab[:].opt()])

            # ================= layer 1 =================
            prop(ts0)                     # acc0 = A @ ts0
            scale_acc()                   # acc0 = t1
            make_tsn()                    # tsn = ts1 = d * t1
            fm_fill(fmA, 64, "a")
            allgather(tabs[0])
            prop(tabs[0])                 # acc0 = A @ ts1
            scale_acc()                   # acc0 = t2'

            # L1 matmuls + relu; h overwrites fmA[0:64]; also build ts_h
            for j in range(ntile):
                j0, j1 = j * 512, min((j + 1) * 512, PADLOC)
                w = j1 - j0
                b0 = j * 4
                nb = min(4, NCOLS - b0)
                fm2 = fm2p.tile([64, 512], bf16, tag="fm2", name=f"fm2_{j}")
                pt2 = psT.tile([64, 512], f32, tag="pt", name=f"pt2_{j}")
                for bi in range(nb):
                    nc.tensor.transpose(
                        out=pt2[:, bi * 128:(bi + 1) * 128],
                        in_=acc0[:, _cm(b0 + bi), :],
                        identity=identf[:])
                nc.vector.tensor_copy(out=fm2[0:64, :nb * 128],
                                      in_=pt2[:, :nb * 128])
                pm = psM.tile([64, 512], f32, tag="pm", name=f"pm_{j}")
                nc.tensor.matmul(out=pm[:, :w], lhsT=w1a[:], rhs=fmA[:, j0:j1],
                                 start=True, stop=False)
                nc.tensor.matmul(out=pm[:, :w], lhsT=w1b[:], rhs=fm2[0:64, :w],
                                 start=False, stop=True)
                nc.scalar.activation(fmA[0:64, j0:j1], pm[:, :w], Relu,
                                     bias=b1v[:, 0:1])
                # ts_h = d * h into tsn (split order: evens then odds)
                ph = psH.tile([128, 4, CIN], bf16, tag="ph", name=f"ph_{j}")
                perm = [0, 2, 1, 3]
                for bi in range(nb):
                    b = b0 + bi
                    nc.tensor.transpose(
                        out=ph[:, perm[bi], :],
                        in_=fmA[0:64, b * 128:(b + 1) * 128],
                        identity=identb[0:64, 0:64])
                ne = (nb + 1) // 2
                nc.vector.tensor_tensor(
                    out=tsn[:, b0 // 2:b0 // 2 + ne, :], in0=ph[:, 0:ne, :],
                    in1=dnm[:, b0 // 2:b0 // 2 + ne, :]
                        .to_broadcast([128, ne, CIN]),
                    op=MUL)
                no = nb // 2
                if no:
                    nc.vector.tensor_tensor(
                        out=tsn[:, GH + b0 // 2:GH + b0 // 2 + no, :],
                        in0=ph[:, 2:2 + no, :],
                        in1=dnm[:, GH + b0 // 2:GH + b0 // 2 + no, :]
                            .to_broadcast([128, no, CIN]),
                        op=MUL)

            # ================= layer 2 =================
            allgather(tabs[1])
            prop(tabs[1])                 # acc0 = A @ ts_h
            scale_acc()
            make_tsn()
            fm_fill(fmA, 64, "b")
            allgather(tabs[2])
            prop(tabs[2])                 # acc0 = A @ ts1'
            scale_acc()

            for j in range(ntile):
                j0, j1 = j * 512, min((j + 1) * 512, PADLOC)
                w = j1 - j0
                b0 = j * 4
                nb = min(4, NCOLS - b0)
                fm2 = fm2p.tile([64, 512], bf16, tag="fm2", name=f"fm2b_{j}")
                pt2 = psT.tile([64, 512], f32, tag="pt", name=f"pt2b_{j}")
                for bi in range(nb):
                    nc.tensor.transpose(
                        out=pt2[:, bi * 128:(bi + 1) * 128],
                        in_=acc0[:, _cm(b0 + bi), :],
                        identity=identf[:])
                nc.vector.tensor_copy(out=fm2[0:64, :nb * 128],
                                      in_=pt2[:, :nb * 128])
                pm = psM.tile([64, 512], f32, tag="pm", name=f"pmb_{j}")
                nc.tensor.matmul(out=pm[0:COUT, :w], lhsT=w2a[:],
                                 rhs=fmA[:, j0:j1], start=True, stop=False)
                nc.tensor.matmul(out=pm[0:COUT, :w], lhsT=w2b[:],
                                 rhs=fm2[0:64, :w], start=False, stop=True)
                ofm = ofmp.tile([COUT, 512], f32, tag="ofm", name=f"ofm_{j}")
                nc.scalar.activation(ofm[:, :w], pm[0:COUT, :w], Ident,
                                     bias=b2v[:, 0:1])
                nc.sync.dma_start(out=out_d[:, j0:j1], in_=ofm[:, :w])

    nc.finalize()
    return nc


# --------------------------------------------------------------------------
# PJRT runner (jit once, reuse)
# --------------------------------------------------------------------------

def _make_runner(nc):
    import jax
    from jax.sharding import Mesh, PartitionSpec
    from jax.experimental.shard_map import shard_map
    from concourse import mybir
    from concourse.bass2jax import (_bass_exec_p, install_neuronx_cc_hook,
                                    partition_id_tensor)

    install_neuronx_cc_hook()
    partition_name = nc.partition_id_tensor.name if nc.partition_id_tensor else None
    in_names, out_names, out_avals = [], [], []
    for alloc in nc.m.functions[0].allocations:
        if not isinstance(alloc, mybir.MemoryLocationSet):
            continue
        name = alloc.memorylocations[0].name
        if alloc.kind == "ExternalInput":
            if name != partition_name:
                in_names.append(name)
        elif alloc.kind == "ExternalOutput":
            out_names.append(name)
            out_avals.append(jax.core.ShapedArray(tuple(alloc.tensor_shape),
                                                  mybir.dt.np(alloc.dtype)))
    n_params = len(in_names)
    all_in = list(in_names) + list(out_names)
    if partition_name is not None:
        all_in.append(partition_name)
    donate = tuple(range(n_params, n_params + len(out_names)))

    def _body(*args):
        operands = list(args)
        if partition_name is not None:
            operands.append(partition_id_tensor())
        return tuple(_bass_exec_p.bind(
            *operands, out_avals=tuple(out_avals), in_names=tuple(all_in),
            out_names=tuple(out_names), lowering_input_output_aliases=(),
            sim_require_finite=True, sim_require_nnan=True, nc=nc))

    devices = jax.devices()[:NCORES]
    mesh = Mesh(np.asarray(devices), ("core",))
    in_specs = (PartitionSpec("core"),) * (n_params + len(out_names))
    out_specs = (PartitionSpec("core"),) * len(out_names)
    fn = jax.jit(shard_map(_body, mesh=mesh, in_specs=in_specs,
                           out_specs=out_specs, check_rep=False),
                 donate_argnums=donate, keep_unused=True)

    state = {"staged": None}

    def stage(in_maps):
        per_core = [[np.asarray(m[n]) for n in in_names] for m in in_maps]
        concat_in = [np.concatenate([per_core[c][i] for c in range(NCORES)],
                                    axis=0) for i in range(n_params)]
        state["staged"] = [jax.device_put(a) for a in concat_in]
        jax.block_until_ready(state["staged"])

    def run():
        import time
        concat_zeros = [np.zeros((NCORES * a.shape[0], *a.shape[1:]), a.dtype)
                        for a in out_avals]
        zs = [jax.device_put(z) for z in concat_zeros]
        jax.block_until_ready(zs)
        t0 = time.time()
        outs = fn(*state["staged"], *zs)
        jax.block_until_ready(outs)
        dt = time.time() - t0
        res = [{n: np.asarray(outs[i]).reshape(NCORES, *out_avals[i].shape)[c]
                for i, n in enumerate(out_names)} for c in range(NCORES)]
        return res, dt

    return stage, run


# --------------------------------------------------------------------------
# Entry point
# --------------------------------------------------------------------------

def _get_compiled(edge_index):
    key = hash(np.asarray(edge_index)[:, :: max(1, N_EDGES // 1024)].tobytes())
    if key in _CACHE:
        return _CACHE[key]
    prep = _prep_indices(edge_index)
    nc = _build_program(prep["sched"], prep["nslot"])
    stage, run = _make_runner(nc)
    _CACHE[key] = (prep, stage, run)
    return _CACHE[key]


def kernel(x, edge_index, W1, b1, W2, b2):
    x = np.asarray(x, dtype=np.float32)
    W1 = np.asarray(W1, dtype=np.float32)
    W2 = np.asarray(W2, dtype=np.float32)
    b1 = np.asarray(b1, dtype=np.float32)
    b2 = np.asarray(b2, dtype=np.float32)

    prep, stage, run = _get_compiled(edge_index)
    lnode, dloc, d = prep["lnode"], prep["dloc"], prep["d"]

    # gather table for prop 1: pos-ordered d*x with zero rows
    ts0 = np.zeros((NTAB, CIN), dtype=np.float32)
    dx = d[:, None] * x
    for c in range(NCORES):
        ts0[c * TSTRIDE:c * TSTRIDE + NLOC] = dx[lnode[c]]

    w1a = _bf16(np.concatenate([W1[0] - W1[2], -W1[1]], axis=0))   # [128, 64]
    w1b = _bf16(2.0 * W1[2])                                       # [64, 64]
    w2a = _bf16(np.concatenate([W2[0] - W2[2], -W2[1]], axis=0))   # [128, 32]
    w2b = _bf16(2.0 * W2[2])                                       # [64, 32]
    identf = np.eye(128, dtype=np.float32)
    identb = _bf16(identf)

    in_maps = []
    for c in range(NCORES):
        xl = np.zeros((PADLOC, CIN), dtype=np.float32)
        xl[:NLOC] = x[lnode[c]]
        dn = dloc[c].reshape(NCOLS, 128).T                 # [128, NCOLS]
        dnm2 = np.zeros((128, 2 * GH), dtype=np.float32)
        dnm2[:, 0:GH - 1] = dn[:, 0:NCOLS:2]               # even columns
        dnm2[:, GH:2 * GH - 1] = dn[:, 1:NCOLS:2]          # odd columns
        in_maps.append({
            "ts0": ts0, "xfm": _bf16(np.ascontiguousarray(xl.T)),
            "dnm2": dnm2, "gixs": prep["gw"][c], "sixs": prep["sw"][c],
            "w1a": w1a, "w1b": w1b, "w2a": w2a, "w2b": w2b,
            "b1v": b1[:, None], "b2v": b2[:, None],
            "identf": identf, "identb": identb,
        })

    stage(in_maps)
    res, dt = run()
    kernel.last_exec_wall_s = dt
    kernel.rerun = run

    out = np.empty((N_NODES, COUT), dtype=np.float32)
    for c in range(NCORES):
        out[lnode[c]] = np.ascontiguousarray(res[c]["out"].T)[:NLOC]
    return out


# revision 15
# speedup vs baseline: 3.5859x; 3.5859x over previous
"""ChebNet (K=3, 2 layers) message-passing kernel for 8 Trainium2 NeuronCores.

Strategy (dest-sharded; batched dma_gather + CCE dma_scatter_add):
  - Nodes are sharded across 8 cores by contiguous id range (12500 each).
  - Within a core, destination nodes are sorted by in-degree (descending):
    rank r -> grid slot r (column b = r//128, partition p = r%128).
  - The propagation  L_hat @ t = -D^-1/2 A D^-1/2 t  is factored as
    -d * (A @ (d * t)): only an unweighted gather+sum over edges remains;
    scalings/signs fold into per-node d multiplies and host-prepared weights.
  - A @ ts per core: edge source rows are fetched with `dma_gather` (one
    descriptor per edge, int16 indices into one of 4 windows of the
    replicated fp32 table) into SBUF chunk buffers, then `dma_scatter_add`
    CCE-accumulates each edge row into the destination accumulator in SBUF
    (parity-split layout: even columns in half A, odd in half B).  Scatter
    calls are split by edge occurrence within (dest, window) so no
    destination repeats inside a call (CCE read-modify-write stays
    race-free).  ~25 gather + ~60 scatter calls per propagation replace the
    ~1200 per-column indirect-DMA chains of the old scheme (whose per-call
    SWDGE overhead dominated at ~7 us/call).
  - Full scaled node features are replicated on every core via AllGather
    between propagations (fp32 tables; dma_gather requires 256 B rows).
  - ChebConv matmuls run in bf16 with K packed to 128 ([T0;T1] against
    [W0-W2;-W1]) plus a K=64 term (2*W2), fp32 PSUM accumulation.
  - Output is written feature-major [COUT, PADLOC]; the host transposes.

Self-contained: hardcodes the problem shapes; builds indices from the given
edge_index on the host (numpy), compiles one SPMD Bass program, runs it on
cores 0..7, and reassembles the full [100000, 32] output.
"""

import numpy as np

N_NODES = 100_000
N_EDGES = 1_200_000
CIN, CHID, COUT, KCH = 64, 64, 32, 3
NCORES = 8
NLOC = N_NODES // NCORES            # 12500
NCOLS = (NLOC + 127) // 128         # 98
PADLOC = NCOLS * 128                # 12544
TSTRIDE = PADLOC + 16               # per-core block in gather tables
NTAB = TSTRIDE * NCORES             # 100480
WIN = 2 * TSTRIDE                   # 25120 rows: int16-addressable window
NWIN = 4
ZLOC = PADLOC                       # zero row inside each window (block pad)
DUMP = PADLOC                       # dest slots >= DUMP land in dump column
CH = 4096                           # max slots per gather chunk / scatter run
GH = NCOLS // 2 + 1                 # acc half-columns incl dump (50)

_CACHE = {}


def _bf16(a):
    import ml_dtypes
    return np.asarray(a, dtype=ml_dtypes.bfloat16)


def _wrap16(vals):
    """int16 index list (len%16==0) -> [128, n/16] wrapped+replicated."""
    n = len(vals)
    assert n % 16 == 0
    return np.tile(vals.reshape(n // 16, 16).T, (8, 1))


# --------------------------------------------------------------------------
# Host-side index construction
# --------------------------------------------------------------------------

def _prep_indices(edge_index):
    row = np.asarray(edge_index[0], dtype=np.int64)
    col = np.asarray(edge_index[1], dtype=np.int64)
    deg = np.bincount(row, minlength=N_NODES)
    d = np.where(deg > 0, 1.0 / np.sqrt(np.maximum(deg, 1)), 0.0).astype(np.float32)

    # per-core degree-descending permutation
    rank = np.empty(N_NODES, dtype=np.int64)
    lnode = np.empty((NCORES, NLOC), dtype=np.int64)
    for c in range(NCORES):
        ids = np.arange(c * NLOC, (c + 1) * NLOC)
        order = np.argsort(-deg[ids], kind="stable")
        lnode[c] = ids[order]
        rank[ids[order]] = np.arange(NLOC)
    pos = (np.arange(N_NODES) // NLOC) * TSTRIDE + rank   # table position

    # per-edge: dest core/rank, source window/local row
    dpos = pos[row]
    core_e = dpos // TSTRIDE
    r_e = dpos % TSTRIDE                   # dest rank (< NLOC)
    spos = pos[col]
    w_e = spos // WIN
    s_e = (spos % WIN).astype(np.int64)    # source row within window

    # window-degree profile per (core, window, rank); schedule = max over cores
    # n_cnt[c][w][k] = #dests on core c with window-degree > k
    wdeg = np.zeros((NCORES, NWIN, NLOC), dtype=np.int64)
    for c in range(NCORES):
        for w in range(NWIN):
            m = (core_e == c) & (w_e == w)
            wdeg[c, w] = np.bincount(r_e[m], minlength=NLOC)
    maxk = int(wdeg.max())
    # counts[c, w, k]
    counts = np.zeros((NCORES, NWIN, maxk), dtype=np.int64)
    for c in range(NCORES):
        for w in range(NWIN):
            for k in range(maxk):
                counts[c, w, k] = int((wdeg[c, w] > k).sum())
    n_sched = counts.max(axis=0)           # [NWIN, maxk]

    # run pieces per (w, k): lengths (%128, split at CH)
    # piece list per window, in k order; then chunks greedy-pack pieces <= CH
    def r128(x):
        return (x + 127) // 128 * 128

    pieces = {w: [] for w in range(NWIN)}  # (k, piece_off_in_run, L)
    for w in range(NWIN):
        for k in range(maxk):
            L = r128(int(n_sched[w, k]))
            if L == 0:
                continue
            off = 0
            while off < L:
                pieces[w].append((k, off, min(CH, L - off)))
                off += CH

    # chunks: (w, n, [piece...]), slot ranges; gather/scatter col offsets
    sched = []          # list of dicts
    gcols = scols = 0
    slot_base = {}      # (w, k) -> global slot base of the run
    slot_cursor = 0
    run_counter = 0
    for w in range(NWIN):
        cur = None
        for (k, off, L) in pieces[w]:
            if cur is None or cur["n"] + L > CH:
                if cur is not None:
                    sched.append(cur)
                cur = {"w": w, "n": 0, "gc0": gcols, "runs": [],
                       "slot0": slot_cursor}
                gcols_start = gcols
            if off == 0:
                slot_base[(w, k)] = slot_cursor
            cur["runs"].append({"a": cur["n"], "n": L, "sc0": scols,
                                "set": run_counter % 2})
            run_counter += 1
            cur["n"] += L
            gcols += L // 16
            scols += L // 16
            slot_cursor += L
        if cur is not None:
            sched.append(cur)
    nslot = slot_cursor

    # per-core index streams
    gixs = np.full((NCORES, nslot), ZLOC, dtype=np.int16)
    sixs = np.empty((NCORES, nslot), dtype=np.int16)
    sixs[:] = (DUMP + np.arange(nslot) % 128).astype(np.int16)
    for c in range(NCORES):
        for w in range(NWIN):
            m = (core_e == c) & (w_e == w)
            r = r_e[m]
            s = s_e[m]
            o = np.argsort(r, kind="stable")
            r, s = r[o], s[o]
            first = np.ones(len(r), dtype=bool)
            first[1:] = r[1:] != r[:-1]
            gs = np.maximum.accumulate(
                np.where(first, np.arange(len(r)), 0))
            occ = np.arange(len(r)) - gs
            # position within run k = index among qualifying ranks, which is
            # the sort-order of r among edges with the same occ
            o2 = np.lexsort((r, occ))
            r2, s2, occ2 = r[o2], s[o2], occ[o2]
            firstk = np.ones(len(r2), dtype=bool)
            firstk[1:] = occ2[1:] != occ2[:-1]
            ks = np.maximum.accumulate(
                np.where(firstk, np.arange(len(r2)), 0))
            posk = np.arange(len(r2)) - ks
            base = np.array([slot_base[(w, int(kk))] for kk in occ2])
            slots = base + posk
            gixs[c, slots] = s2.astype(np.int16)
            sixs[c, slots] = r2.astype(np.int16)

    # wrap the streams per call
    gw = np.empty((NCORES, 128, nslot // 16), dtype=np.int16)
    sw = np.empty((NCORES, 128, nslot // 16), dtype=np.int16)
    for c in range(NCORES):
        for ck in sched:
            s0, n, gc0 = ck["slot0"], ck["n"], ck["gc0"]
            gw[c][:, gc0:gc0 + n // 16] = _wrap16(gixs[c, s0:s0 + n])
            for rn in ck["runs"]:
                a, nr, sc0 = rn["a"], rn["n"], rn["sc0"]
                sw[c][:, sc0:sc0 + nr // 16] = _wrap16(
                    sixs[c, s0 + a:s0 + a + nr])

    dloc = np.zeros((NCORES, PADLOC), dtype=np.float32)
    for c in range(NCORES):
        dloc[c, :NLOC] = d[lnode[c]]

    return {
        "lnode": lnode, "gw": gw, "sw": sw, "sched": sched, "nslot": nslot,
        "dloc": dloc, "d": d,
    }


# --------------------------------------------------------------------------
# Device program
# --------------------------------------------------------------------------

def _cm(b):
    """column -> acc/tsn split-layout index (evens | odds, dump at 49/99)."""
    return b // 2 if b % 2 == 0 else GH + b // 2


def _build_program(sched, nslot):
    from collections import deque
    from concourse import bass, bacc, tile, mybir

    f32 = mybir.dt.float32
    bf16 = mybir.dt.bfloat16
    i16 = mybir.dt.int16
    nc = bacc.Bacc("TRN2", target_bir_lowering=False, debug=False,
                   num_devices=NCORES)

    ts0b = nc.dram_tensor("ts0b", [TSTRIDE, CIN], bf16, kind="ExternalInput")
    xfm = nc.dram_tensor("xfm", [CIN, PADLOC], bf16, kind="ExternalInput")
    dnm_in = nc.dram_tensor("dnm2", [128, 2 * GH], f32, kind="ExternalInput")
    gix_in = nc.dram_tensor("gixs", [16, nslot // 16], i16,
                            kind="ExternalInput")
    six_in = nc.dram_tensor("sixs", [16, nslot // 16], i16,
                            kind="ExternalInput")
    w1a_in = nc.dram_tensor("w1a", [128, CHID], bf16, kind="ExternalInput")
    w1b_in = nc.dram_tensor("w1b", [64, CHID], bf16, kind="ExternalInput")
    w2a_in = nc.dram_tensor("w2a", [128, COUT], bf16, kind="ExternalInput")
    w2b_in = nc.dram_tensor("w2b", [64, COUT], bf16, kind="ExternalInput")
    b1_in = nc.dram_tensor("b1v", [CHID, 1], f32, kind="ExternalInput")
    b2_in = nc.dram_tensor("b2v", [COUT, 1], f32, kind="ExternalInput")
    idf_in = nc.dram_tensor("identf", [128, 128], f32, kind="ExternalInput")
    idb_in = nc.dram_tensor("identb", [128, 128], bf16, kind="ExternalInput")
    out_d = nc.dram_tensor("out", [COUT, PADLOC], f32, kind="ExternalOutput")

    Relu = mybir.ActivationFunctionType.Relu
    Ident = mybir.ActivationFunctionType.Identity
    MUL = mybir.AluOpType.mult
    ADD = mybir.AluOpType.add
    BYP = mybir.AluOpType.bypass

    ntile = (PADLOC + 511) // 512    # 25 matmul tiles of 512 nodes

    with tile.TileContext(nc) as tc:
        with (
            tc.tile_pool(name="consts", bufs=1) as consts,
            tc.tile_pool(name="accp", bufs=1) as accp,
            tc.tile_pool(name="fmp", bufs=1) as fmp,
            tc.tile_pool(name="gp", bufs=3) as gp,
            tc.tile_pool(name="fm2p", bufs=3) as fm2p,
            tc.tile_pool(name="ofmp", bufs=3) as ofmp,
            tc.tile_pool(name="psT", bufs=3, space="PSUM") as psT,
            tc.tile_pool(name="psM", bufs=2, space="PSUM") as psM,
            tc.tile_pool(name="psH", bufs=3, space="PSUM") as psH,
            tc.tile_pool(name="dram", bufs=1, space="DRAM") as dram,
        ):
            # ---- constants into SBUF (index streams replicated to 8 Q7 rows)
            gixs_sb = consts.tile([128, nslot // 16], i16)
            sixs_sb = consts.tile([128, nslot // 16], i16)
            for r in range(8):
                nc.sync.dma_start(out=gixs_sb[16 * r:16 * (r + 1), :],
                                  in_=gix_in[:])
                nc.sync.dma_start(out=sixs_sb[16 * r:16 * (r + 1), :],
                                  in_=six_in[:])
            dnm = consts.tile([128, 2 * GH, 1], f32)
            nc.sync.dma_start(out=dnm[:],
                              in_=dnm_in[:].rearrange("p (b o) -> p b o", o=1))
            dnm_bc = dnm[:].to_broadcast([128, 2 * GH, CIN])
            w1a = consts.tile([128, CHID], bf16)
            nc.sync.dma_start(out=w1a[:], in_=w1a_in[:])
            w1b = consts.tile([64, CHID], bf16)
            nc.sync.dma_start(out=w1b[:], in_=w1b_in[:])
            w2a = consts.tile([128, COUT], bf16)
            nc.sync.dma_start(out=w2a[:], in_=w2a_in[:])
            w2b = consts.tile([64, COUT], bf16)
            nc.sync.dma_start(out=w2b[:], in_=w2b_in[:])
            b1v = consts.tile([CHID, 1], f32)
            nc.sync.dma_start(out=b1v[:], in_=b1_in[:])
            b2v = consts.tile([COUT, 1], f32)
            nc.sync.dma_start(out=b2v[:], in_=b2_in[:])
            identf = consts.tile([128, 128], f32)
            nc.sync.dma_start(out=identf[:], in_=idf_in[:])
            identb = consts.tile([128, 128], bf16)
            nc.sync.dma_start(out=identb[:], in_=idb_in[:])
            zrow = consts.tile([16, CIN], f32)
            nc.gpsimd.memset(zrow[:], 0.0)

            # fmA: partitions 0:64 = Tx0 features (x / h), 64:128 = Tx1_s
            fmA = fmp.tile([128, PADLOC], bf16)
            nc.sync.dma_start(out=fmA[0:64, :], in_=xfm[:])
            # split-ordered (evens|odds) staging for AllGather inputs
            tsn = fmp.tile([128, 2 * GH, CIN], f32)
            # accumulator sets, split-ordered; two sets for scatter overlap
            acc0 = accp.tile([128, 2 * GH, CIN], f32)
            acc1 = accp.tile([128, 2 * GH, CIN], f32)

            # DRAM: AllGather bounce + tables
            ag_in = dram.tile([TSTRIDE, CIN], f32, name="ag_in")
            tabs = [dram.tile([NTAB, CIN], f32, name=f"tab{i}",
                              addr_space="Shared") for i in range(4)]
            nc.sync.dma_start(out=ag_in[PADLOC:TSTRIDE, :], in_=zrow[:])

            def prop(table):
                nc.vector.memset(acc0[:], 0.0)
                nc.vector.memset(acc1[:], 0.0)

                def scatters(g, ck):
                    for rn in ck["runs"]:
                        acc = acc0 if rn["set"] == 0 else acc1
                        a, nr, sc0 = rn["a"], rn["n"], rn["sc0"]
                        nc.gpsimd.dma_scatter_add(
                            out_ap=acc[:, 0:GH, :],
                            in_ap=g[:, a // 128:(a + nr) // 128, :],
                            idxs_ap=sixs_sb[:, sc0:sc0 + nr // 16],
                            num_idxs=nr, num_idxs_reg=nr, elem_size=CIN,
                            sbuf_tokens_per_rank=128, parity_reg=0,
                            out_ap_other=acc[:, GH:2 * GH, :],
                            single_packet=False)

                pend = deque()
                for j, ck in enumerate(sched):
                    g = gp.tile([128, CH // 128, CIN], f32, tag="gg",
                                name=f"g_{j}")
                    w, n, gc0 = ck["w"], ck["n"], ck["gc0"]
                    nc.gpsimd.dma_gather(
                        out_ap=g[:, :n // 128, :],
                        in_ap=table[w * WIN:(w + 1) * WIN, :],
                        idxs_ap=gixs_sb[:, gc0:gc0 + n // 16],
                        num_idxs=n, num_idxs_reg=n, elem_size=CIN,
                        single_packet=False)
                    pend.append((g, ck))
                    if len(pend) == 3:
                        scatters(*pend.popleft())
                while pend:
                    scatters(*pend.popleft())
                # combine sets
                nc.vector.tensor_tensor(out=acc0[:], in0=acc0[:],
                                        in1=acc1[:], op=ADD)

            def scale_acc():
                nc.vector.tensor_tensor(out=acc0[:], in0=acc0[:], in1=dnm_bc,
                                        op=MUL)

            def make_tsn():
                nc.vector.tensor_tensor(out=tsn[:], in0=acc0[:], in1=dnm_bc,
                                        op=MUL)

            def fm_fill(dst, dst_part0, tag):
                """acc0 (=t1) columns -> dst[dst_part0:+64] feature-major."""
                for q in range(0, NCOLS, 4):
                    nb = min(4, NCOLS - q)
                    pt = psT.tile([64, 512], f32, tag="pt",
                                  name=f"pt{tag}_{q}")
                    for bi in range(nb):
                        nc.tensor.transpose(out=pt[:, bi * 128:(bi + 1) * 128],
                                            in_=acc0[:, _cm(q + bi), :],
                                            identity=identf[:])
                    nc.vector.tensor_copy(
                        out=dst[dst_part0:dst_part0 + 64,
                                q * 128:(q + nb) * 128],
                        in_=pt[:, :nb * 128])

            def allgather(tab):
                # evens: table rows 2g*128+p ; odds: rows (2g+1)*128+p
                av = ag_in[0:PADLOC, :].rearrange("(g t p) c -> t p g c",
                                                  t=2, p=128)
                nc.sync.dma_start(out=av[0], in_=tsn[:, 0:GH - 1, :])
                nc.sync.dma_start(out=av[1], in_=tsn[:, GH:2 * GH - 1, :])
                nc.gpsimd.collective_compute(
                    "AllGather", BYP,
                    replica_groups=[list(range(NCORES))],
                    ins=[ag_in[:].opt()], outs=[tab[:].opt()])

            # ================= layer 1 =================
            # build the ts0 table on device: cast local bf16 block to fp32
            # into the AG bounce, then AllGather
            nc.gpsimd.dma_start(out=ag_in[0:PADLOC, :], in_=ts0b[0:PADLOC, :])
            nc.gpsimd.collective_compute(
                "AllGather", BYP, replica_groups=[list(range(NCORES))],
                ins=[ag_in[:].opt()], outs=[tabs[3][:].opt()])
            prop(tabs[3])                 # acc0 = A @ ts0
            scale_acc()                   # acc0 = t1
            make_tsn()                    # tsn = ts1 = d * t1
            fm_fill(fmA, 64, "a")
            allgather(tabs[0])
            prop(tabs[0])                 # acc0 = A @ ts1
            scale_acc()                   # acc0 = t2'

            # L1 matmuls + relu; h overwrites fmA[0:64]; also build ts_h
            for j in range(ntile):
                j0, j1 = j * 512, min((j + 1) * 512, PADLOC)
                w = j1 - j0
                b0 = j * 4
                nb = min(4, NCOLS - b0)
                fm2 = fm2p.tile([64, 512], bf16, tag="fm2", name=f"fm2_{j}")
                pt2 = psT.tile([64, 512], f32, tag="pt", name=f"pt2_{j}")
                for bi in range(nb):
                    nc.tensor.transpose(
                        out=pt2[:, bi * 128:(bi + 1) * 128],
                        in_=acc0[:, _cm(b0 + bi), :],
                        identity=identf[:])
                nc.vector.tensor_copy(out=fm2[0:64, :nb * 128],
                                      in_=pt2[:, :nb * 128])
                pm = psM.tile([64, 512], f32, tag="pm", name=f"pm_{j}")
                nc.tensor.matmul(out=pm[:, :w], lhsT=w1a[:], rhs=fmA[:, j0:j1],
                                 start=True, stop=False)
                nc.tensor.matmul(out=pm[:, :w], lhsT=w1b[:], rhs=fm2[0:64, :w],
                                 start=False, stop=True)
                nc.scalar.activation(fmA[0:64, j0:j1], pm[:, :w], Relu,
                                     bias=b1v[:, 0:1])
                # ts_h = d * h into tsn (split order: evens then odds)
                ph = psH.tile([128, 4, CIN], bf16, tag="ph", name=f"ph_{j}")
                perm = [0, 2, 1, 3]
                for bi in range(nb):
                    b = b0 + bi
                    nc.tensor.transpose(
                        out=ph[:, perm[bi], :],
                        in_=fmA[0:64, b * 128:(b + 1) * 128],
                        identity=identb[0:64, 0:64])
                ne = (nb + 1) // 2
                nc.vector.tensor_tensor(
                    out=tsn[:, b0 // 2:b0 // 2 + ne, :], in0=ph[:, 0:ne, :],
                    in1=dnm[:, b0 // 2:b0 // 2 + ne, :]
                        .to_broadcast([128, ne, CIN]),
                    op=MUL)
                no = nb // 2
                if no:
                    nc.vector.tensor_tensor(
                        out=tsn[:, GH + b0 // 2:GH + b0 // 2 + no, :],
                        in0=ph[:, 2:2 + no, :],
                        in1=dnm[:, GH + b0 // 2:GH + b0 // 2 + no, :]
                            .to_broadcast([128, no, CIN]),
                        op=MUL)

            # ================= layer 2 =================
            allgather(tabs[1])
            prop(tabs[1])                 # acc0 = A @ ts_h
            scale_acc()
            make_tsn()
            fm_fill(fmA, 64, "b")
            allgather(tabs[2])
            prop(tabs[2])                 # acc0 = A @ ts1'
            scale_acc()

            for j in range(ntile):
                j0, j1 = j * 512, min((j + 1) * 512, PADLOC)
                w = j1 - j0
                b0 = j * 4
                nb = min(4, NCOLS - b0)
                fm2 = fm2p.tile([64, 512], bf16, tag="fm2", name=f"fm2b_{j}")
                pt2 = psT.tile([64, 512], f32, tag="pt", name=f"pt2b_{j}")
                for bi in range(nb):
                    nc.tensor.transpose(
                        out=pt2[:, bi * 128:(bi + 1) * 128],
                        in_=acc0[:, _cm(b0 + bi), :],
                        identity=identf[:])
                nc.vector.tensor_copy(out=fm2[0:64, :nb * 128],
                                      in_=pt2[:, :nb * 128])
                pm = psM.tile([64, 512], f32, tag="pm", name=f"pmb_{j}")
                nc.tensor.matmul(out=pm[0:COUT, :w], lhsT=w2a[:],
                                 rhs=fmA[:, j0:j1], start=True, stop=False)
                nc.tensor.matmul(out=pm[0:COUT, :w], lhsT=w2b[:],
                                 rhs=fm2[0:64, :w], start=False, stop=True)
                ofm = ofmp.tile([COUT, 512], f32, tag="ofm", name=f"ofm_{j}")
                nc.scalar.activation(ofm[:, :w], pm[0:COUT, :w], Ident,
                                     bias=b2v[:, 0:1])
                nc.sync.dma_start(out=out_d[:, j0:j1], in_=ofm[:, :w])

    nc.finalize()
    return nc


# --------------------------------------------------------------------------
# PJRT runner (jit once, reuse)
# --------------------------------------------------------------------------

def _make_runner(nc):
    import jax
    from jax.sharding import Mesh, PartitionSpec
    from jax.experimental.shard_map import shard_map
    from concourse import mybir
    from concourse.bass2jax import (_bass_exec_p, install_neuronx_cc_hook,
                                    partition_id_tensor)

    install_neuronx_cc_hook()
    partition_name = nc.partition_id_tensor.name if nc.partition_id_tensor else None
    in_names, out_names, out_avals = [], [], []
    for alloc in nc.m.functions[0].allocations:
        if not isinstance(alloc, mybir.MemoryLocationSet):
            continue
        name = alloc.memorylocations[0].name
        if alloc.kind == "ExternalInput":
            if name != partition_name:
                in_names.append(name)
        elif alloc.kind == "ExternalOutput":
            out_names.append(name)
            out_avals.append(jax.core.ShapedArray(tuple(alloc.tensor_shape),
                                                  mybir.dt.np(alloc.dtype)))
    n_params = len(in_names)
    all_in = list(in_names) + list(out_names)
    if partition_name is not None:
        all_in.append(partition_name)
    donate = tuple(range(n_params, n_params + len(out_names)))

    def _body(*args):
        operands = list(args)
        if partition_name is not None:
            operands.append(partition_id_tensor())
        return tuple(_bass_exec_p.bind(
            *operands, out_avals=tuple(out_avals), in_names=tuple(all_in),
            out_names=tuple(out_names), lowering_input_output_aliases=(),
            sim_require_finite=True, sim_require_nnan=True, nc=nc))

    devices = jax.devices()[:NCORES]
    mesh = Mesh(np.asarray(devices), ("core",))
    in_specs = (PartitionSpec("core"),) * (n_params + len(out_names))
    out_specs = (PartitionSpec("core"),) * len(out_names)
    fn = jax.jit(shard_map(_body, mesh=mesh, in_specs=in_specs,
                           out_specs=out_specs, check_rep=False),
                 donate_argnums=donate, keep_unused=True)

    state = {"staged": None}

    def stage(in_maps):
        per_core = [[np.asarray(m[n]) for n in in_names] for m in in_maps]
        concat_in = [np.concatenate([per_core[c][i] for c in range(NCORES)],
                                    axis=0) for i in range(n_params)]
        state["staged"] = [jax.device_put(a) for a in concat_in]
        jax.block_until_ready(state["staged"])

    def run():
        import time
        concat_zeros = [np.zeros((NCORES * a.shape[0], *a.shape[1:]), a.dtype)
                        for a in out_avals]
        zs = [jax.device_put(z) for z in concat_zeros]
        jax.block_until_ready(zs)
        t0 = time.time()
        outs = fn(*state["staged"], *zs)
        jax.block_until_ready(outs)
        dt = time.time() - t0
        res = [{n: np.asarray(outs[i]).reshape(NCORES, *out_avals[i].shape)[c]
                for i, n in enumerate(out_names)} for c in range(NCORES)]
        return res, dt

    return stage, run


# --------------------------------------------------------------------------
# Entry point
# --------------------------------------------------------------------------

def _get_compiled(edge_index):
    key = hash(np.asarray(edge_index)[:, :: max(1, N_EDGES // 1024)].tobytes())
    if key in _CACHE:
        return _CACHE[key]
    prep = _prep_indices(edge_index)
    nc = _build_program(prep["sched"], prep["nslot"])
    stage, run = _make_runner(nc)
    _CACHE[key] = (prep, stage, run)
    return _CACHE[key]


def kernel(x, edge_index, W1, b1, W2, b2):
    x = np.asarray(x, dtype=np.float32)
    W1 = np.asarray(W1, dtype=np.float32)
    W2 = np.asarray(W2, dtype=np.float32)
    b1 = np.asarray(b1, dtype=np.float32)
    b2 = np.asarray(b2, dtype=np.float32)

    prep, stage, run = _get_compiled(edge_index)
    lnode, dloc, d = prep["lnode"], prep["dloc"], prep["d"]

    # per-core ts0 block: pos-ordered d*x with zero pad rows (bf16)
    dx = d[:, None] * x
    ts0b = []
    for c in range(NCORES):
        blk = np.zeros((TSTRIDE, CIN), dtype=np.float32)
        blk[:NLOC] = dx[lnode[c]]
        ts0b.append(_bf16(blk))

    w1a = _bf16(np.concatenate([W1[0] - W1[2], -W1[1]], axis=0))   # [128, 64]
    w1b = _bf16(2.0 * W1[2])                                       # [64, 64]
    w2a = _bf16(np.concatenate([W2[0] - W2[2], -W2[1]], axis=0))   # [128, 32]
    w2b = _bf16(2.0 * W2[2])                                       # [64, 32]
    identf = np.eye(128, dtype=np.float32)
    identb = _bf16(identf)

    in_maps = []
    for c in range(NCORES):
        xl = np.zeros((PADLOC, CIN), dtype=np.float32)
        xl[:NLOC] = x[lnode[c]]
        dn = dloc[c].reshape(NCOLS, 128).T                 # [128, NCOLS]
        dnm2 = np.zeros((128, 2 * GH), dtype=np.float32)
        dnm2[:, 0:GH - 1] = dn[:, 0:NCOLS:2]               # even columns
        dnm2[:, GH:2 * GH - 1] = dn[:, 1:NCOLS:2]          # odd columns
        in_maps.append({
            "ts0b": ts0b[c], "xfm": _bf16(np.ascontiguousarray(xl.T)),
            "dnm2": dnm2, "gixs": prep["gw"][c][:16], "sixs": prep["sw"][c][:16],
            "w1a": w1a, "w1b": w1b, "w2a": w2a, "w2b": w2b,
            "b1v": b1[:, None], "b2v": b2[:, None],
            "identf": identf, "identb": identb,
        })

    stage(in_maps)
    res, dt = run()
    kernel.last_exec_wall_s = dt
    kernel.rerun = run

    out = np.empty((N_NODES, COUT), dtype=np.float32)
    for c in range(NCORES):
        out[lnode[c]] = np.ascontiguousarray(res[c]["out"].T)[:NLOC]
    return out
